# revision 4
# baseline (speedup 1.0000x reference)
"""Bilateral denoiser (11x11) v2 — 8 TRN2 cores, PE-accumulate design.

Sharding: 8 cores = 4 images x 2 column halves (512 cols, full 1024 rows).
Per core: 4 mega-tiles of 256 rows; 128 partitions/tile, each owning a
32x32 pixel block with a 5-px halo in the free dim (plane rows HC=48).

Per tap pair t=(dy,dx), d=|t| (fp32 dot / fp16 downstream):
  dote = clip(n.n_shift, 1e-4, 1)        fp32  DVE(+Pool m2/m1)
  lnd  = ln(dote)                        fp16  ACT
  adz  = |z_shift - z|                   fp16  Pool sub + ACT abs
  rp   = min(recip_dz/(128 d), 1e4/128)  fp16  DVE TS (per distance)
  per tap s in {+t,-t}:
    u = adz_s * rp; v = lnd_s - u        fp16  DVE TT 2x
    w = exp(128 v - d^2/8)               fp16  ACT -> d4[3N:4N)
    d4[0:3N) = col_s * w                 fp16  DVE TT 2x (bcast w)
    psum[128,4N] += I @ d4               fp32  PE (8 matmuls FD=512)
Center tap w=1: d4 = [col_center, ones], start=True resets PSUM.
Epilogue: inv = 1/psum_w (ACT recip), out = psum_c * inv (DVE), DMA out.
"""

import sys

for _p in ("/root/.axon_site", "/root/.axon_site/_ro/trn_rl_repo",
           "/root/.axon_site/_ro/pypackages", "/opt/trn_rl_repo", "/opt/pypackages"):
    if _p not in sys.path:
        sys.path.append(_p)

import math
import numpy as np

B, H, W = 4, 1024, 1024
RAD = 5
NCORES = 8
COLS = 512
RB, CW = 32, 32        # block rows/cols per partition
NP2 = RB * CW          # 1024 pixels per partition per tile
HB, HC = RB + 2 * RAD, 48          # halo'd plane 42 x 48 (2 pad cols)
NSH = HB * HC                      # 2016
EB, EC = RB + RAD, 44              # expanded frame 37 x 44
NEXF = EB * EC                     # 1628
NTILES = 4
GR, GC = 8, 16         # partition grid per tile (8 row-blocks x 16 col-blocks)
ZPAD = 6.0e4
CLIP_LO = 1.0e-4
RCAP = 1.0e4 / 128.0
RECIP_PAD = 1.0e9

POOL_M2 = True     # z-channel normal product on Pool
POOL_M1 = True     # y-channel normal product on Pool
POOL_ZDIFF = True  # depth diff on Pool
USE_ADDCLIP = False  # fused add+clip custom DVE op: walrus codegen rejects it
WEIGHTLESS_MM = False  # IR requires 2-input matmuls; dedup pass strips reloads

_cache = {}


def _positive_half_taps():
    taps = []
    for dy in range(0, RAD + 1):
        for dx in range(-RAD, RAD + 1):
            if dy == 0 and dx <= 0:
                continue
            taps.append((dy, dx))
    taps.sort(key=lambda t: (t[0] * t[0] + t[1] * t[1], t))
    return taps


def _register_add_clip():
    """out = clip(in0 + in1, s0, s1) fused DVE op."""
    import numpy as _np
    from concourse import dve_ops as dvo
    from concourse.dve_spec import Spec, Src0, Src1, C0, C1, minn, maxx, lower
    from concourse.dve_uop import DveOpSpec

    if hasattr(dvo, "ANT_ADD_CLIP_X"):
        return dvo.ANT_ADD_CLIP_X
    name = "ANT_ADD_CLIP_X"
    spec = Spec(
        body=minn(maxx(Src0 + Src1, C0), C1),
        reference=lambda in0, in1, s0, s1, imm2: _np.minimum(
            _np.maximum(in0.astype(_np.float32) + in1, s0), s1
        ),
    )
    row = dvo._CUSTOM_DVE_ROW_BASE + len(dvo.OPS)
    dvo._SUB_OPCODE_FOR_NAME[name] = row
    shas = {}
    for ver in ("v3", "v4"):
        try:
            uops = lower(spec, ver=ver)
            shas[ver] = DveOpSpec(
                name=name, opcode=row, uops=uops, rd1_en=True
            ).sha(ver)
        except Exception:
            pass
    op = dvo.DveOp(name, spec, subdim=False, uops_sha=shas)
    dvo.OPS.append(op)
    dvo.CUSTOM_DVE_SPECS[name] = spec
    dvo.ANT_ADD_CLIP_X = op
    return op


def _patch_tile(tile, mybir):
    def _patched_drain_and_barrier(self, tick_clock, wait_clock):
        from bass_rust import ScopedClock

        probe = mybir.InstDrain(
            name="wait-probe", engine=mybir.EngineType.SP, ins=[], outs=[]
        )
        wait_clock.add_sem_waits(probe, ScopedClock({None: tick_clock.global_clock}))
        si = probe.sync_info
        waits = list(si.on_wait) if si is not None else []
        handles = {h.num: h for h in self.sems.allocated().values()}
        for wt in waits:
            assert wt.wait_reg is None and wt.wait_mode == "sem-ge-imm", wt
            self.nc.sync.wait_ge(handles[wt.id], wt.wait_value)
        self.nc.sync.drain()
        self.nc.all_engine_barrier()
        popped = self.nc._tile_sem_poison_stack.pop()
        assert popped is self._sem_poison
        self.nc.clear_and_free_semaphores(list(self.sems.allocated().values()))
        self.nc.all_engine_barrier()

    tile.TileContext._drain_and_barrier = _patched_drain_and_barrier


def _dedup_ldweights(nc, mybir):
    """Identity weights are loaded once; later reloads are redundant.
    Drop sync-free InstLdweights after the first (weights persist in PE)."""
    removed = 0
    for blk in nc.main_func.blocks:
        il = blk.instructions
        seen = False
        k = 0
        while k < len(il):
            inst = il[k]
            if isinstance(inst, mybir.InstLdweights):
                si = inst.sync_info
                clean = si is None or (not si.on_wait and not si.on_update)
                if seen and clean:
                    del il[k]
                    removed += 1
                    continue
                seen = True
            k += 1
    return removed


def _split_multi_waits(nc, mybir):
    total = 0
    for blk in nc.main_func.blocks:
        il = blk.instructions
        k = 0
        while k < len(il):
            inst = il[k]
            si = inst.sync_info
            if si is not None and si.on_wait and len(si.on_wait) > 1:
                waits = list(si.on_wait)
                for j, wt in enumerate(waits[:-1]):
                    nop = mybir.InstNoOp(
                        name=f"{inst.name}-hw{j}", engine=inst.engine, ins=[], outs=[]
                    )
                    nop.sync_info = mybir.SyncInfo(on_wait=[wt], on_update=[])
                    nc.register_instruction(nop, overwrite=True)
                    il.insert(k, nop)
                    k += 1
                inst.sync_info = mybir.SyncInfo(
                    on_wait=[waits[-1]], on_update=list(si.on_update or [])
                )
                total += 1
            k += 1
    return total


def _build_program(ntiles=NTILES, npairs=None):
    import concourse.bass as bass
    import concourse.mybir as mybir
    import concourse.tile as tile

    f32 = mybir.dt.float32
    f16 = mybir.dt.float16
    Alu = mybir.AluOpType
    Act = mybir.ActivationFunctionType

    _patch_tile(tile, mybir)
    addclip = _register_add_clip() if USE_ADDCLIP else None

    nc = bass.Bass("TRN2")

    taps = _positive_half_taps()
    if npairs is not None:
        taps = taps[:npairs]
    dsqs = sorted({dy * dy + dx * dx for dy, dx in taps})
    for v in [0.0] + [-(d2) / 8.0 for d2 in dsqs]:
        key = (f32, float(v))
        if key not in nc.const_aps.aps:
            t = nc.alloc_sbuf_tensor(f"cbias_{abs(v):.4f}".replace(".", "_"),
                                     [128, 1], f32)
            nc.gpsimd.memset(t.ap(), float(v))
            nc.const_aps.aps[key] = t.ap()
    nc.all_engine_barrier()

    nrm_t = nc.dram_tensor("nrm", [ntiles, 128, 3 * NSH], f32, kind="ExternalInput")
    col_t = nc.dram_tensor("col", [ntiles, 128, 3 * NSH], f16, kind="ExternalInput")
    z_t = nc.dram_tensor("z", [ntiles, 128, NSH], f32, kind="ExternalInput")
    rdz_t = nc.dram_tensor("rdz", [ntiles, 128, NP2], f32, kind="ExternalInput")
    id_t = nc.dram_tensor("ident", [128, 128], f16, kind="ExternalInput")
    on_t = nc.dram_tensor("onesw", [128, NP2], f16, kind="ExternalInput")
    out_t = nc.dram_tensor("out", [ntiles, 128, 3 * NP2], f16, kind="ExternalOutput")

    def mkap(base, dims, extra_off=0):
        a = base.copy()
        a.ap = mybir.VecI64Pair(dims)
        a.offset = a.offset + extra_off
        return a

    with tile.TileContext(nc) as tc:
        with tc.tile_pool(name="stat", bufs=1) as stat_pool, \
             tc.tile_pool(name="pln", bufs=2) as pln_pool, \
             tc.tile_pool(name="e0p", bufs=1) as e0_pool, \
             tc.tile_pool(name="e12p", bufs=1) as e12_pool, \
             tc.tile_pool(name="lnp", bufs=2) as ln_pool, \
             tc.tile_pool(name="ezp", bufs=2) as ez_pool, \
             tc.tile_pool(name="uvp", bufs=2) as uv_pool, \
             tc.tile_pool(name="rpp", bufs=2) as rp_pool, \
             tc.tile_pool(name="d4p", bufs=3) as d4_pool, \
             tc.tile_pool(name="d3p", bufs=3) as d3_pool, \
             tc.tile_pool(name="cen", bufs=1) as cen_pool, \
             tc.tile_pool(name="ps", bufs=1, space="PSUM") as ps_pool:
            ident = stat_pool.tile([128, 128], f16, tag="identity")
            ones = stat_pool.tile([128, NP2], f16, tag="onesw")
            nc.sync.dma_start(ident, id_t.ap())
            nc.sync.dma_start(ones, on_t.ap())
            for T in range(ntiles):
                nrm = pln_pool.tile([128, 3 * NSH], f32, tag="nrm")
                col = pln_pool.tile([128, 3 * NSH], f16, tag="col")
                zz = pln_pool.tile([128, NSH], f32, tag="zz")
                rdz = pln_pool.tile([128, NP2], f32, tag="rdz")
                out3 = cen_pool.tile([128, 3 * NP2], f16, tag="out3")
                d4c = cen_pool.tile([128, 3 * NP2], f16, tag="d4c")
                inv = cen_pool.tile([128, NP2], f32, tag="inv")
                acc = ps_pool.tile([128, 4 * NP2], f32, tag="acc")

                nc.sync.dma_start(nrm, nrm_t.ap()[T])
                nc.scalar.dma_start(col, col_t.ap()[T])
                nc.gpsimd.dma_start(zz, z_t.ap()[T])
                nc.gpsimd.dma_start(rdz, rdz_t.ap()[T])

                def nv(c, y0, ny, x0, nx, _t=nrm):
                    ps = _t.ap[0][0]
                    off = c * NSH + (y0 + RAD) * HC + (x0 + RAD)
                    return mkap(_t, [[ps, 128], [HC, ny], [1, nx]], off)

                def zv(y0, ny, x0, nx, _t=zz):
                    ps = _t.ap[0][0]
                    off = (y0 + RAD) * HC + (x0 + RAD)
                    return mkap(_t, [[ps, 128], [HC, ny], [1, nx]], off)

                def col3v(y0, x0, _t=col):
                    ps = _t.ap[0][0]
                    off = (y0 + RAD) * HC + (x0 + RAD)
                    return mkap(
                        _t, [[ps, 128], [NSH, 3], [HC, RB], [1, CW]], off
                    )

                def ev(t, y0, ny, x0, nx):
                    ps = t.ap[0][0]
                    off = (y0 + RAD) * EC + (x0 + RAD + 1)
                    return mkap(t, [[ps, 128], [EC, ny], [1, nx]], off)

                def flat(t, n, off=0):
                    return mkap(t, [[t.ap[0][0], 128], [1, n]], off)

                def pe_mm(out_ap, rhs_ap, start, stop):
                    if not WEIGHTLESS_MM:
                        nc.tensor.matmul(out_ap, ident, rhs_ap,
                                         start=start, stop=stop)
                        return
                    eng = nc.tensor
                    inst = mybir.InstMatmult(
                        name=nc.get_next_instruction_name(),
                        replication_resolution=0,
                        replication_shift_amnt=0,
                        replication_num_rows=0,
                        start_tensor_calc=start,
                        stop_tensor_calc=stop,
                        ins=[eng.lower_ap(rhs_ap)],
                        outs=[eng.lower_ap(out_ap)],
                        perf_mode=None,
                        is_transpose=None,
                        ifmap_quant_offset=None,
                        weights_quant_offset=None,
                        bass_skip_group_check=True,
                        tile_position=(0, 0),
                        tile_size=(128, 128),
                    )
                    eng.add_instruction(inst)

                if WEIGHTLESS_MM:
                    nc.tensor.ldweights(ident)

                # ---- center tap: w = 1 (unit normals), PSUM reset ----
                nc.vector.tensor_copy(flat(d4c, 3 * NP2), col3v(0, 0))
                for j in range(6):
                    pe_mm(flat(acc, 512, 512 * j),
                          flat(d4c, 512, 512 * j), True, False)
                for j in range(2):
                    pe_mm(flat(acc, 512, 3 * NP2 + 512 * j),
                          flat(ones, 512, 512 * j), True, False)

                # ---- pipelined pair loop ----
                state = {"d2": None, "rp": None}

                def emit_pairlevel(pair):
                    dy, dx = pair
                    ylo, NY = -dy, RB + dy
                    xlo = -dx if dx > 0 else 0
                    XW = CW + abs(dx)
                    if XW % 2:
                        XW += 1
                    e01 = e0_pool.tile([128, 2 * NEXF], f32, tag="e0")
                    e2 = e12_pool.tile([128, NEXF], f32, tag="e12")
                    lnd = ln_pool.tile([128, NEXF], f16, tag="lnd")
                    ez = ez_pool.tile([128, NEXF], f16, tag="ez")
                    args = (ylo, NY, xlo, XW)

                    def cen(c):
                        return nv(c, ylo, NY, xlo, XW)

                    def shv(c):
                        return nv(c, ylo + dy, NY, xlo + dx, XW)

                    # Pool: m2 first (the DVE add blocks on it mid-pair),
                    # depth diff second (consumed later, via ACT abs)
                    m2 = ev(e2, *args)
                    (nc.gpsimd if POOL_M2 else nc.vector).tensor_tensor(
                        m2, cen(2), shv(2), Alu.mult)
                    zd = ev(ez, *args)
                    if POOL_ZDIFF:
                        nc.gpsimd.tensor_tensor(
                            zd, zv(ylo + dy, NY, xlo + dx, XW),
                            zv(ylo, NY, xlo, XW), Alu.subtract)
                    else:
                        nc.vector.tensor_tensor(
                            zd, zv(ylo + dy, NY, xlo + dx, XW),
                            zv(ylo, NY, xlo, XW), Alu.subtract)
                    # DVE: ch0+ch1 products in one stacked op, then folds
                    eoff = (ylo + RAD) * EC + (xlo + RAD + 1)
                    c01 = mkap(nrm, [[nrm.ap[0][0], 128], [NSH, 2],
                                     [HC, NY], [1, XW]],
                               (ylo + RAD) * HC + (xlo + RAD))
                    s01 = mkap(nrm, [[nrm.ap[0][0], 128], [NSH, 2],
                                     [HC, NY], [1, XW]],
                               (ylo + dy + RAD) * HC + (xlo + dx + RAD))
                    m01 = mkap(e01, [[e01.ap[0][0], 128], [NEXF, 2],
                                     [EC, NY], [1, XW]], eoff)
                    m1 = mkap(e01, [[e01.ap[0][0], 128], [EC, NY], [1, XW]],
                              eoff)
                    m1b = mkap(e01, [[e01.ap[0][0], 128], [EC, NY], [1, XW]],
                               NEXF + eoff)
                    nc.vector.tensor_tensor(m01, c01, s01, Alu.mult)
                    nc.vector.tensor_tensor(m1, m1, m1b, Alu.add)
                    if USE_ADDCLIP:
                        nc.vector._custom_dve(
                            addclip, out=m2, in0=m2, in1=m1,
                            s0=CLIP_LO, s1=1.0,
                        )
                    else:
                        nc.vector.tensor_tensor(m2, m2, m1, Alu.add)
                        nc.vector.tensor_scalar(
                            m2, m2, CLIP_LO, 1.0, Alu.max, Alu.min)
                    nc.scalar.activation(ev(lnd, *args), m2, Act.Ln)
                    nc.scalar.activation(zd, zd, Act.Abs)
                    return (pair, lnd, ez)

                def emit_taps_a(pl):
                    """u, v, exp for both taps — queues ACT exp ahead of the
                    next pair's ln/abs to avoid DVE stalling on w."""
                    (dy, dx), lnd, ez = pl
                    d2 = dy * dy + dx * dx
                    if state["d2"] != d2:
                        state["d2"] = d2
                        rp = rp_pool.tile([128, NP2], f16, tag="rp")
                        invd = 1.0 / (128.0 * math.sqrt(d2))
                        nc.vector.tensor_scalar(
                            flat(rp, NP2), flat(rdz, NP2), invd, RCAP,
                            Alu.mult, Alu.min)
                        state["rp"] = rp
                    rp = state["rp"]
                    # stacked halves: 0 = mirror (-dy,-dx), 1 = plus (+dy,+dx)
                    dlt = dy * EC + dx
                    offm = (RAD - dy) * EC + (RAD + 1 - dx)
                    u2 = uv_pool.tile([128, 2 * NP2], f16, tag="u")
                    v2 = uv_pool.tile([128, 2 * NP2], f16, tag="v")
                    w2 = d4_pool.tile([128, 2 * NP2], f16, tag="w2")

                    def stk(t, off0):
                        return mkap(t, [[t.ap[0][0], 128], [dlt, 2],
                                        [EC, RB], [1, CW]], off0)

                    def half2(t):
                        return mkap(t, [[t.ap[0][0], 128], [NP2, 2],
                                        [CW, RB], [1, CW]], 0)

                    nc.vector.tensor_tensor(
                        half2(u2), stk(ez, offm),
                        mkap(rp, [[rp.ap[0][0], 128], [0, 2],
                                  [CW, RB], [1, CW]]), Alu.mult)
                    nc.vector.tensor_tensor(
                        half2(v2), stk(lnd, offm), half2(u2), Alu.subtract)
                    nc.scalar.activation(
                        flat(w2, 2 * NP2), flat(v2, 2 * NP2), Act.Exp,
                        bias=-(d2) / 8.0, scale=128.0)
                    return w2

                def emit_taps_b(pl, w2, last):
                    (dy, dx), lnd, ez = pl
                    for s, (sdy, sdx) in ((0, (-dy, -dx)), (1, (dy, dx))):
                        d3 = d3_pool.tile([128, 3 * NP2], f16, tag="d3")
                        wb = mkap(w2, [[w2.ap[0][0], 128], [0, 3],
                                       [CW, RB], [1, CW]], s * NP2)
                        d3v = mkap(d3, [[d3.ap[0][0], 128], [NP2, 3],
                                        [CW, RB], [1, CW]], 0)
                        nc.vector.tensor_tensor(d3v, col3v(sdy, sdx), wb,
                                                Alu.mult)
                        stop = last and s == 1
                        for j in range(6):
                            pe_mm(flat(acc, 512, 512 * j),
                                  flat(d3, 512, 512 * j), False, stop)
                        for j in range(2):
                            pe_mm(flat(acc, 512, 3 * NP2 + 512 * j),
                                  flat(w2, 512, s * NP2 + 512 * j),
                                  False, stop)

                npair = len(taps)
                pls = [None] * npair
                pls[0] = emit_pairlevel(taps[0])
                for i in range(npair):
                    w2t = emit_taps_a(pls[i])
                    if i + 1 < npair:
                        pls[i + 1] = emit_pairlevel(taps[i + 1])
                    emit_taps_b(pls[i], w2t, last=(i == npair - 1))

                # ---- epilogue ----
                nc.vector.reciprocal(flat(inv, NP2), flat(acc, NP2, 3 * NP2))
                ib = mkap(inv, [[inv.ap[0][0], 128], [0, 3], [CW, RB],
                                [1, CW]], 0)
                a3 = mkap(acc, [[acc.ap[0][0], 128], [NP2, 3], [CW, RB],
                                [1, CW]], 0)
                o3 = mkap(out3, [[out3.ap[0][0], 128], [NP2, 3], [CW, RB],
                                 [1, CW]], 0)
                nc.vector.tensor_tensor(o3, a3, ib, Alu.mult)
                nc.sync.dma_start(out_t.ap()[T], out3)
    _dedup_ldweights(nc, mybir)
    _split_multi_waits(nc, mybir)
    return nc


def _prep_inputs(col, nrm, zdz, ntiles=NTILES):
    from numpy.lib.stride_tricks import sliding_window_view

    GH, GW = H + 2 * RAD, COLS + RAD + HC  # 1034 x 565 padded plane (idx = x+6)
    nrm_in = np.empty((NCORES, ntiles, 128, 3 * NSH), np.float32)
    col_in = np.empty((NCORES, ntiles, 128, 3 * NSH), np.float16)
    z_in = np.empty((NCORES, ntiles, 128, NSH), np.float32)
    rdz_in = np.empty((NCORES, ntiles, 128, NP2), np.float32)
    rstarts = RB * np.arange(GR)
    cstarts = CW * np.arange(GC)
    for c in range(NCORES):
        b, h = divmod(c, 2)
        c0 = 512 * h
        # global padded planes: rows idx y+5, cols idx x+6 (x core-local)
        nrmp = np.zeros((3, GH, GW), np.float32)
        colp = np.zeros((3, GH, GW), np.float16)
        zp = np.full((GH, GW), ZPAD, np.float32)
        gxs = max(0, c0 - 6)
        gxe = min(W, c0 + COLS + HC - 6)
        js = gxs - (c0 - 6)
        je = js + (gxe - gxs)
        sl = np.s_[RAD: RAD + H, js:je]
        for k in range(3):
            nrmp[k][sl] = nrm[b, :, gxs:gxe, k]
            colp[k][sl] = col[b, :, gxs:gxe, k].astype(np.float16)
        zp[sl] = zdz[b, :, gxs:gxe, 0]
        with np.errstate(divide="ignore"):
            rdzc = 1.0 / np.maximum(zdz[b, :, c0:c0 + COLS, 1], 0.0)
        for T in range(ntiles):
            ys = 256 * T + rstarts
            for k in range(3):
                swn = sliding_window_view(nrmp[k], (HB, HC))
                blk = swn[ys][:, cstarts + 1]  # A-phase: g = x0+1
                nrm_in[c, T, :, k * NSH:(k + 1) * NSH] = blk.reshape(128, NSH)
                swc = sliding_window_view(colp[k], (HB, HC))
                blk = swc[ys][:, cstarts + 1]
                col_in[c, T, :, k * NSH:(k + 1) * NSH] = blk.reshape(128, NSH)
            swz = sliding_window_view(zp, (HB, HC))
            blk = swz[ys][:, cstarts + 1]
            z_in[c, T] = blk.reshape(128, NSH)
            d = rdzc[256 * T: 256 * (T + 1)].reshape(GR, RB, GC, CW)
            rdz_in[c, T] = d.transpose(0, 2, 1, 3).reshape(128, NP2)
    return nrm_in, col_in, z_in, rdz_in


def _gather_output(results, ntiles=NTILES):
    out = np.empty((B, H, W, 3), np.float32)
    for c in range(NCORES):
        b, h = divmod(c, 2)
        o = results[c]["out"]  # [ntiles, 128, 3*NP2]
        o = o.reshape(ntiles, GR, GC, 3, RB, CW)
        img = o.transpose(3, 0, 1, 4, 2, 5).reshape(3, ntiles * 256, COLS)
        out[b, : ntiles * 256, 512 * h: 512 * h + COLS, :] = np.moveaxis(
            img, 0, -1)
    return out


def kernel(col, nrm, zdz):
    from concourse import bass_utils

    if "nc" not in _cache:
        _cache["nc"] = _build_program()
    nc = _cache["nc"]
    nrm_in, col_in, z_in, rdz_in = _prep_inputs(
        np.asarray(col, np.float32), np.asarray(nrm, np.float32),
        np.asarray(zdz, np.float32),
    )
    ident = np.eye(128, dtype=np.float16)
    onesw = np.ones((128, NP2), dtype=np.float16)
    in_maps = [
        {"nrm": nrm_in[c], "col": col_in[c], "z": z_in[c],
         "rdz": rdz_in[c], "ident": ident, "onesw": onesw}
        for c in range(NCORES)
    ]
    res = bass_utils.run_bass_kernel_spmd(nc, in_maps, core_ids=list(range(NCORES)))
    return _gather_output(res.results)
